# revision 65
# baseline (speedup 1.0000x reference)
"""Multi-head attention (QKV proj + RoPE + masked softmax + out-proj) on 8 TRN2 cores.

Sharding (tensor-parallel heads x data-parallel batch):
  core c in 0..7  ->  batch b = c // 4, head-group g = c % 4 (heads 4g..4g+3).
Each core computes its 512-wide q/k/v head slice, RoPE, attention for its 4
heads, and a partial output projection  ao_slice @ Wo[:, slice].T  (full [S, D]).
Host sums the 4 partials per batch and applies the final clip.

Device layouts (per core):
  xT    [D, S]  bf16   x[b].T (clipped on device unless host proves |x|<CLAMP)
  wqkvT [D, 3*GD] bf16 concat of Wq/Wk/Wv head-slice transposes; resident in SBUF
  woT   [512, D] bf16  Wo[:, slice].T
  cosT  [128, S] f32; sinM [128, S] f32 (sign/swap-folded rope table)
  q/k kept d-major [128(d), S] per head; v kept s-major [128(s), 512(hd)]
  scores computed transposed [sk, sq] so softmax denom = ones-matmul on PE.

Program order interleaves qkv(j+1) / attn(j) / oproj(j-1) so the Tile
scheduler can fill attention-pipeline stalls with projection/output matmuls.
"""

import os
import sys

if "/opt/trn_rl_repo" not in sys.path:
    sys.path.insert(0, "/opt/trn_rl_repo")
os.environ.setdefault("JAX_PLATFORMS", "")

from contextlib import ExitStack

import ml_dtypes
import numpy as np

import concourse.bass as bass
import concourse.mybir as mybir
import concourse.tile as tile
from concourse import bacc
from concourse.bass_utils import run_bass_kernel_spmd

BF16 = ml_dtypes.bfloat16
B, S, D, H = 2, 2048, 2048, 16
DH = 128
CLAMP = 10.0
SCALE = float(1.0 / np.sqrt(np.float32(DH)))
NCORES = 8
GH = 4            # heads per core
GD = GH * DH      # 512
SB = 512          # s-block width
NSB = S // SB     # 4
NE = D // 128     # 16 contraction chunks
NSK = S // 128    # 16
F32 = mybir.dt.float32
BF = mybir.dt.bfloat16
MIN_ = mybir.AluOpType.min
MAX_ = mybir.AluOpType.max
MULT = mybir.AluOpType.mult
EXP = mybir.ActivationFunctionType.Exp
EXPHI = float(np.exp(np.float32(CLAMP)))
EXPLO = float(np.exp(np.float32(-CLAMP)))

# module-level knobs read by test.py
TRACE = False
TRACE_DIR = None
LAST_EXEC_NS = None
LAST_RESULT = None

_PROGRAMS = {}

# tunables (read at program-build time)
KNOBS = {
    "px": 2,
    "prope": 3,
    "prot": 2,
    "pnrm": 2,
    "po_st": 6,
    "pp": 10,
}


def _build_program(variant, no_xclip=False, no_expclip=False):
    """variant: 'causal' (tril mask), 'ones' (no mask), 'general' (mask tensor).

    no_xclip: host verified max|x| < CLAMP, so clip(x) is an identity.
    no_expclip: host verified (exact candidate-pair check) that all scores in
    the live (unmasked) region stay inside +-CLAMP, so the score clip is an
    identity and exp never saturates."""
    nc = bacc.Bacc(
        "TRN2",
        target_bir_lowering=False,
        debug=False,
        enable_asserts=False,
        num_devices=NCORES,
    )
    # flat layouts so whole tensors load in a few large DMAs:
    #   xTr[p, ((j*NE)+e)*SB + c]        = x.T[e*128+p, j*SB+c]
    #   wq/wkT[p, d*(NE*128) + e*128 + c] = W.T[e*128+p, d*128+c]  (d-major: one
    #     512KB slice per q/k chain, so chain d can start before slice d+1 lands)
    #   wvT[p, e*GD + c]                 = Wv.T[e*128+p, c]
    xTr = nc.dram_tensor("xTr", [128, NSB * NE * SB], BF, kind="ExternalInput")
    wqT = nc.dram_tensor("wqT", [128, NE * GD], BF, kind="ExternalInput")
    wkT = nc.dram_tensor("wkT", [128, NE * GD], BF, kind="ExternalInput")
    wvT = nc.dram_tensor("wvT", [128, NE * GD], BF, kind="ExternalInput")
    woT = nc.dram_tensor("woT", [GD, D], BF, kind="ExternalInput")
    cosT = nc.dram_tensor("cosT", [DH, S], BF, kind="ExternalInput")
    sinM = nc.dram_tensor("sinM", [DH, S], BF, kind="ExternalInput")
    bandT = maskT = selT = None
    if variant == "causal":
        bandT = nc.dram_tensor("bandT", [128, 896], BF, kind="ExternalInput")
        # selT[p, h*128+o] = 1/32 if 32h <= p < 32h+32 else 0 — broadcast matrix
        # that expands the col-tiled denominators to all 128 partitions
        selT = nc.dram_tensor("selT", [128, GD], F32, kind="ExternalInput")
    elif variant == "general":
        maskT = nc.dram_tensor("maskT", [S, S], BF, kind="ExternalInput")
    outp = nc.dram_tensor("outp", [S, D], BF, kind="ExternalOutput")

    with ExitStack() as ctx:
        tc = ctx.enter_context(tile.TileContext(nc))
        p_x = ctx.enter_context(tc.tile_pool(name="px", bufs=KNOBS["px"]))
        p_w = ctx.enter_context(tc.tile_pool(name="pw", bufs=1))
        p_qk = ctx.enter_context(tc.tile_pool(name="pqk", bufs=2 * GH))
        p_v = ctx.enter_context(tc.tile_pool(name="pv", bufs=NSK))
        p_rope = ctx.enter_context(tc.tile_pool(name="prope", bufs=KNOBS["prope"]))
        p_rot = ctx.enter_context(tc.tile_pool(name="prot", bufs=KNOBS["prot"]))
        p_tab = ctx.enter_context(tc.tile_pool(name="ptab", bufs=1))
        p_p = ctx.enter_context(tc.tile_pool(name="pp", bufs=KNOBS["pp"]))
        p_ao = ctx.enter_context(tc.tile_pool(name="pao", bufs=GH))
        p_nrm = ctx.enter_context(tc.tile_pool(name="pnrm", bufs=KNOBS["pnrm"]))
        p_wo = ctx.enter_context(tc.tile_pool(name="pwo", bufs=GH))
        p_o = ctx.enter_context(tc.tile_pool(name="po_st", bufs=KNOBS["po_st"]))
        # PSUM: 8 banks split per phase/tag so cross-phase interleave never
        # serializes on a shared slot ring. bufs is per-tag: 2+2+1+2+1 = 8.
        p_ps_q = ctx.enter_context(tc.tile_pool(name="ppsq", bufs=2, space="PSUM"))
        p_ps_po = ctx.enter_context(tc.tile_pool(name="ppspo", bufs=2, space="PSUM"))
        p_ps_pd = ctx.enter_context(tc.tile_pool(name="ppspd", bufs=1, space="PSUM"))
        p_ps_sc = ctx.enter_context(tc.tile_pool(name="ppssc", bufs=2, space="PSUM"))
        p_ps_o = ctx.enter_context(tc.tile_pool(name="ppso", bufs=1, space="PSUM"))
        if variant == "general":
            p_m = ctx.enter_context(tc.tile_pool(name="pm", bufs=NSK + 4))

        dmaq = [nc.sync, nc.scalar, nc.gpsimd]

        # persistent SBUF tensors (flat: slice [:, e*GD+...] per e-chunk)
        wqr = p_w.tile([128, NE * GD], BF, tag="wq", name="wqr")
        wkr = p_w.tile([128, NE * GD], BF, tag="wk", name="wkr")
        wvr = p_w.tile([128, NE * GD], BF, tag="wv", name="wvr")
        qbf = [p_qk.tile([128, S], BF, tag="qk", name=f"qbf{i}") for i in range(GH)]
        kbf = [p_qk.tile([128, S], BF, tag="qk", name=f"kbf{i}") for i in range(GH)]
        aobf = [p_ao.tile([128, S], BF, tag="ao", name=f"aobf{i}") for i in range(GH)]
        vbf = [None] * NSK
        xts = [None] * NSB
        wot = []

        def x_load(j, q, widths=None):
            t = p_x.tile([128, NE * SB], BF, tag="x", name=f"xt{j}")
            widths = widths or [NE * SB]
            off = 0
            for w in widths:
                q.dma_start(
                    out=t[:, off : off + w],
                    in_=xTr[:, j * NE * SB + off : j * NE * SB + off + w],
                )
                off += w
            assert off == NE * SB
            if not no_xclip:
                nc.gpsimd.tensor_scalar(t, t, CLAMP, -CLAMP, MIN_, MAX_)
            xts[j] = t

        # ---------------- QKV projections + RoPE ----------------
        # rings: list of (pool, tag) cycled for the chain PSUM tiles. Blocks
        # that don't overlap attention borrow its idle banks for deeper
        # drain pipelining.
        def qkv_block(j, rings):
            cs = slice(j * SB, (j + 1) * SB)
            xt = xts[j]
            ci = 0
            # q, k: d-major [128(d=head), s 512]; one PSUM chain per head
            for wbuf, dstbuf in ((wqr, qbf), (wkr, kbf)):
                for d_ in range(GH):
                    pool, ptag = rings[ci % len(rings)]
                    ci += 1
                    ps = pool.tile([128, SB], F32, tag=ptag, name="psqk")
                    d0 = d_ * NE * 128
                    for e in range(NE):
                        nc.tensor.matmul(
                            ps,
                            lhsT=wbuf[:, d0 + e * 128 : d0 + (e + 1) * 128],
                            rhs=xt[:, e * SB : (e + 1) * SB],
                            start=(e == 0),
                            stop=(e == NE - 1),
                        )
                    # drain+clip on DVE (frees the PSUM bank fast); rope math
                    # in bf16 for 2x DVE rate
                    qc = p_rope.tile([128, SB], BF, tag="r1", name="qc")
                    nc.vector.tensor_scalar(qc, ps, CLAMP, -CLAMP, MIN_, MAX_)
                    # rotate_half via DMA (cross-partition moves are free on DMA);
                    # keep these off the scalar queue so ACT stays free for exp
                    qcr = p_rot.tile([128, SB], BF, tag="rot", name="qcr")
                    rq = nc.sync if d_ % 2 else nc.gpsimd
                    rq.dma_start(out=qcr[0:64, :], in_=qc[64:128, :])
                    rq.dma_start(out=qcr[64:128, :], in_=qc[0:64, :])
                    t2 = p_rope.tile([128, SB], BF, tag="r2", name="t2")
                    nc.vector.tensor_tensor(t2, qcr, sinm[:, cs], MULT)
                    qp = p_rope.tile([128, SB], BF, tag="r3", name="qp")
                    nc.vector.tensor_tensor(qp, qc, cosc[:, cs], MULT)
                    nc.vector.tensor_add(qp, qp, t2)
                    nc.gpsimd.tensor_scalar(
                        dstbuf[d_][:, cs], qp, CLAMP, -CLAMP, MIN_, MAX_
                    )

            # v: s-major [s_tile 128, hd 512]
            for st in range(4):
                pool, ptag = rings[ci % len(rings)]
                ci += 1
                vps = pool.tile([128, GD], F32, tag=ptag, name="vps")
                for e in range(NE):
                    nc.tensor.matmul(
                        vps,
                        lhsT=xt[:, e * SB + st * 128 : e * SB + (st + 1) * 128],
                        rhs=wvr[:, e * GD : (e + 1) * GD],
                        start=(e == 0),
                        stop=(e == NE - 1),
                    )
                vt = p_v.tile([128, GD], BF, tag="v", name=f"vt{j}_{st}")
                nc.vector.tensor_scalar(vt, vps, CLAMP, -CLAMP, MIN_, MAX_)
                vbf[j * 4 + st] = vt

        # ---------------- attention ----------------
        def attn_block(j):
            cs = slice(j * SB, (j + 1) * SB)
            nsk = 4 * j + 4 if variant == "causal" else NSK
            mts = None
            if variant == "general":
                mts = []
                for sk in range(NSK):
                    mt = p_m.tile([128, SB], BF, tag="m", name=f"mt{sk}")
                    nc.sync.dma_start(
                        out=mt, in_=maskT[sk * 128 : (sk + 1) * 128, cs]
                    )
                    mts.append(mt)
            for h in range(GH):
                po = p_ps_po.tile([128, SB], F32, tag="po", name="po")
                pd = p_ps_pd.tile([128, SB], F32, tag="pd", name="pd")
                for sk in range(nsk):
                    # causal diag tiles: columns < 128*r are fully masked; skip them
                    c0 = 0
                    if variant == "causal" and sk >= 4 * j:
                        c0 = 128 * (sk - 4 * j)
                    w_ = SB - c0
                    psc = p_ps_sc.tile([128, SB], F32, tag="psc", name="psc")
                    nc.tensor.matmul(
                        psc[:, c0:SB],
                        lhsT=kbf[h][:, sk * 128 : (sk + 1) * 128],
                        rhs=qbf[h][:, j * SB + c0 : (j + 1) * SB],
                        start=True,
                        stop=True,
                    )
                    pt = p_p.tile([128, SB], BF, tag="p", name="pt")
                    nc.scalar.activation(pt[:, c0:SB], psc[:, c0:SB], EXP, scale=SCALE)
                    # post-exp clip == exp of pre-clipped score (exp is monotone;
                    # ACT exp saturates to inf/0 which min/max maps to exp(+-10)).
                    if variant == "general":
                        nc.vector.tensor_scalar(
                            pt[:, c0:SB], pt[:, c0:SB], EXPHI, EXPLO, MIN_, MAX_
                        )
                        nc.vector.tensor_tensor(pt, pt, mts[sk], MULT)
                    elif variant == "causal" and sk >= 4 * j:
                        if no_expclip:
                            nc.vector.tensor_tensor(
                                pt[:, c0:SB], pt[:, c0:SB],
                                band[:, 384 : 384 + w_], MULT,
                            )
                        else:
                            nc.vector.scalar_tensor_tensor(
                                pt[:, c0:SB], pt[:, c0:SB], EXPHI,
                                band[:, 384 : 384 + w_], MIN_, MULT,
                            )
                    elif not no_expclip:
                        nc.vector.tensor_scalar(
                            pt[:, c0:SB], pt[:, c0:SB], EXPHI, EXPLO, MIN_, MAX_
                        )
                    nc.tensor.matmul(
                        po[:, c0:SB],
                        lhsT=vbf[sk][:, h * 128 : (h + 1) * 128],
                        rhs=pt[:, c0:SB],
                        start=(sk == 0),
                        stop=(sk == nsk - 1),
                    )
                    nc.tensor.matmul(
                        pd[:, c0:SB],
                        lhsT=ones,
                        rhs=pt[:, c0:SB],
                        start=(sk == 0),
                        stop=(sk == nsk - 1),
                    )
                # pd rows are all identical (= softmax denominator broadcast)
                rcb = p_nrm.tile([128, SB], F32, tag="rcb", name="rcb")
                nc.vector.reciprocal_approx_fast(rcb, pd)
                a32 = p_nrm.tile([128, SB], F32, tag="a32", name="a32")
                nc.vector.tensor_tensor(a32, po, rcb, MULT)
                nc.gpsimd.tensor_scalar(
                    aobf[h][:, cs], a32, CLAMP, -CLAMP, MIN_, MAX_
                )

        def attn_block_ct(j):
            # causal-only; runs when no qkv block overlaps (PSUM free): sk-outer,
            # heads inner; the 4 heads' softmax denominators ride one PSUM bank
            # via col-tiled ones-matmuls (concurrent 32-col groups), then a
            # broadcast matmul expands 1/d per head to 128 partitions.
            cs = slice(j * SB, (j + 1) * SB)
            nsk = 4 * j + 4
            po_pairs = [
                (p_ps_po, "po"), (p_ps_po, "po"), (p_ps_q, "ps"), (p_ps_q, "ps"),
            ]
            pos = [
                pool.tile([128, SB], F32, tag=ptag, name=f"poct{h}")
                for h, (pool, ptag) in enumerate(po_pairs)
            ]
            pd4 = p_ps_pd.tile([128, SB], F32, tag="pd", name="pd4")
            for sk in range(nsk):
                c0 = 128 * (sk - 4 * j) if sk >= 4 * j else 0
                w_ = SB - c0
                pts = []
                for h in range(GH):
                    psc = p_ps_sc.tile([128, SB], F32, tag="psc", name="psc")
                    nc.tensor.matmul(
                        psc[:, c0:SB],
                        lhsT=kbf[h][:, sk * 128 : (sk + 1) * 128],
                        rhs=qbf[h][:, j * SB + c0 : (j + 1) * SB],
                        start=True,
                        stop=True,
                    )
                    pt = p_p.tile([128, SB], BF, tag="p", name="pt")
                    nc.scalar.activation(pt[:, c0:SB], psc[:, c0:SB], EXP, scale=SCALE)
                    if sk >= 4 * j:
                        if no_expclip:
                            nc.vector.tensor_tensor(
                                pt[:, c0:SB], pt[:, c0:SB],
                                band[:, 384 : 384 + w_], MULT,
                            )
                        else:
                            nc.vector.scalar_tensor_tensor(
                                pt[:, c0:SB], pt[:, c0:SB], EXPHI,
                                band[:, 384 : 384 + w_], MIN_, MULT,
                            )
                    elif not no_expclip:
                        nc.vector.tensor_scalar(
                            pt[:, c0:SB], pt[:, c0:SB], EXPHI, EXPLO, MIN_, MAX_
                        )
                    nc.tensor.matmul(
                        pos[h][:, c0:SB],
                        lhsT=vbf[sk][:, h * 128 : (h + 1) * 128],
                        rhs=pt[:, c0:SB],
                        start=(sk == 0),
                        stop=(sk == nsk - 1),
                    )
                    pts.append(pt)
                # 4 col-tiled denominator matmuls back-to-back -> concurrent
                # 32-col groups. Only the very first clears the bank; later
                # groups' first writes land on cleared has_written bits and
                # overwrite, so every group accumulates correctly.
                for h in range(GH):
                    nc.tensor.matmul(
                        pd4[32 * h : 32 * h + 32, c0:SB],
                        lhsT=ones[:, 0:32],
                        rhs=pts[h][:, c0:SB],
                        start=(sk == 0 and h == 0),
                        stop=(sk == nsk - 1 and h == GH - 1),
                        tile_position=(0, 32 * h),
                        skip_group_check=True,
                    )
            rcp4 = p_nrm.tile([128, SB], F32, tag="rcb", name="rcp4")
            nc.vector.reciprocal_approx_fast(rcp4, pd4)
            for h in range(GH):
                rcbps = p_ps_sc.tile([128, SB], F32, tag="psc", name="rcbps")
                nc.tensor.matmul(
                    rcbps,
                    lhsT=selc[:, h * 128 : (h + 1) * 128],
                    rhs=rcp4,
                    start=True,
                    stop=True,
                )
                rcb = p_nrm.tile([128, SB], F32, tag="a32", name="rcbsb")
                nc.scalar.copy(rcb, rcbps)
                a32 = p_nrm.tile([128, SB], F32, tag="a32", name="a32")
                nc.vector.tensor_tensor(a32, pos[h], rcb, MULT)
                nc.gpsimd.tensor_scalar(
                    aobf[h][:, cs], a32, CLAMP, -CLAMP, MIN_, MAX_
                )

        # -------- output projection (partial over this head slice) --------
        def oproj_block(j, rings, use_act=True):
            # use_act=False while attention runs: ACT must stay free for exp
            ci = 0
            for sq in range(4 * j, 4 * j + 4):
                for eb in range(NSB):
                    pfpool, pftag = rings[ci % len(rings)]
                    ci += 1
                    pf = pfpool.tile([128, SB], F32, tag=pftag, name="pf")
                    for h in range(GH):
                        nc.tensor.matmul(
                            pf,
                            lhsT=aobf[h][:, sq * 128 : (sq + 1) * 128],
                            rhs=wot[h][:, eb * SB : (eb + 1) * SB],
                            start=(h == 0),
                            stop=(h == GH - 1),
                        )
                    ot = p_o.tile([128, SB], BF, tag="ot", name="ot")
                    if use_act and eb % 2 == 0:
                        nc.scalar.copy(ot, pf)
                    else:
                        nc.vector.tensor_copy(ot, pf)
                    nc.sync.dma_start(
                        out=outp[sq * 128 : (sq + 1) * 128, eb * SB : (eb + 1) * SB],
                        in_=ot,
                    )

        # ---------------- program order ----------------
        # HBM transfers share bandwidth round-robin once issued, so strict
        # per-queue FIFO sequencing IS the priority mechanism: critical-path
        # loads (wq, x0) head their queues; deferrables queue strictly behind.
        #   sync:   wq d0..d3 | wk d0..d3 | x2 | outputs
        #   gpsimd: x0 c0..c3 | wv c0..c3 | x1 | wo | x3
        #   scalar: cos sin band (small, needed ~15us in)
        NC4 = 4
        cw = NE * GD // NC4
        # progressive chunks: first pieces small so the first chain's e=0
        # matmul fires as early as possible
        for c0, c1 in ((0, cw // 4), (cw // 4, cw // 2), (cw // 2, cw)):
            nc.sync.dma_start(out=wqr[:, c0:c1], in_=wqT[:, c0:c1])
        for c in range(1, NC4):
            nc.sync.dma_start(
                out=wqr[:, c * cw : (c + 1) * cw], in_=wqT[:, c * cw : (c + 1) * cw]
            )
        x_load(0, nc.gpsimd, widths=[512, 512, 1024, 2048, 4096])

        # wk_d0 right behind wq on sync (k-chains need it ~24us in); rope
        # tables woven after it (consumed a few us later); then the rest of
        # wk. scalar stays clear so only two streams share HBM early.
        nc.sync.dma_start(out=wkr[:, 0:cw], in_=wkT[:, 0:cw])
        cosc = p_tab.tile([DH, S], BF, tag="cos")
        nc.sync.dma_start(out=cosc, in_=cosT[:, :])
        sinm = p_tab.tile([DH, S], BF, tag="sin")
        nc.sync.dma_start(out=sinm, in_=sinM[:, :])
        ones = p_tab.tile([128, 128], BF, tag="ones")
        nc.vector.memset(ones, 1.0)
        for c in range(1, NC4):
            nc.sync.dma_start(
                out=wkr[:, c * cw : (c + 1) * cw], in_=wkT[:, c * cw : (c + 1) * cw]
            )
        band = selc = None
        if variant == "causal":
            band = p_tab.tile([128, 896], BF, tag="band")
            nc.sync.dma_start(out=band, in_=bandT[:, :])
            selc = p_tab.tile([128, GD], F32, tag="sel")
            nc.sync.dma_start(out=selc, in_=selT[:, :])
        for c in range(NC4):
            nc.gpsimd.dma_start(
                out=wvr[:, c * cw : (c + 1) * cw], in_=wvT[:, c * cw : (c + 1) * cw]
            )
        x_load(1, nc.gpsimd, widths=[4096, 4096])

        wide = [(p_ps_q, "ps"), (p_ps_sc, "psc"), (p_ps_po, "po")]
        narrow = [(p_ps_q, "ps")]
        qkv_block(0, wide)
        qkv_block(1, wide)
        attn_block(1)
        for hh in range(GH):
            t = p_wo.tile([128, D], BF, tag="wo", name=f"wot{hh}")
            nc.gpsimd.dma_start(out=t, in_=woT[hh * 128 : (hh + 1) * 128, :])
            wot.append(t)
        x_load(2, nc.sync)
        qkv_block(2, narrow)
        attn_block(2)
        oproj_block(1, [(p_ps_o, "pf")], use_act=False)
        x_load(3, nc.gpsimd)
        qkv_block(3, narrow)
        attn_block(3)
        oproj_block(2, [(p_ps_o, "pf")], use_act=False)
        attn_block(0)
        # qkv ring is idle post-qkv: 2-bank filler for attn(0)'s exp stalls
        oproj_block(
            3, [(p_ps_q, "ps"), (p_ps_o, "pf"), (p_ps_q, "ps")], use_act=False
        )
        # everything else is drained by now: dense multi-bank tail
        oproj_block(0, [(p_ps_q, "ps"), (p_ps_o, "pf"), (p_ps_q, "ps")])

    nc.compile()
    return nc


def _get_program(variant, no_xclip=False, no_expclip=False):
    key = (variant, no_xclip, no_expclip, tuple(sorted(KNOBS.items())))
    if key not in _PROGRAMS:
        _PROGRAMS[key] = _build_program(variant, no_xclip, no_expclip)
    return _PROGRAMS[key]


def _rope_tables():
    inv_freq = 1.0 / (10000.0 ** (np.arange(0, DH, 2, dtype=np.float32) / np.float32(DH)))
    pos = np.arange(S, dtype=np.float32)
    freqs = pos[:, None] * inv_freq[None, :]          # [S, DH/2]
    emb = np.concatenate([freqs, freqs], axis=-1)     # [S, DH]
    return np.cos(emb).astype(np.float32), np.sin(emb).astype(np.float32)


def _rot(t):
    return np.concatenate([-t[..., 64:], t[..., :64]], axis=-1)


def _prove_no_expclip(x, Wq, Wk, causal):
    """Exact host-side proof that no live score reaches +-CLAMP.

    Returns True only if, for every unmasked (q,k) pair, |q.k|*SCALE stays
    below CLAMP with margin for device-side bf16 rounding of x/W/q/k.
    Candidate pairs are prefiltered by the Cauchy-Schwarz norm product and
    then checked with exact dot products."""
    cos_h, sin_h = _rope_tables()
    lim = CLAMP * 0.999
    qrs = []
    for b in range(B):
        xb = x[b].astype(np.float32)
        pair = []
        for W in (Wq, Wk):
            qh = xb @ np.asarray(W, dtype=np.float32).T
            if np.abs(qh).max() >= lim:
                return False  # pre-rope clip binds; bail conservatively
            qh = qh.reshape(S, H, DH)
            qr = qh * cos_h[:, None, :] + _rot(qh) * sin_h[:, None, :]
            if np.abs(qr).max() >= lim:
                return False  # post-rope clip binds; bail conservatively
            pair.append(qr)
        qrs.append(pair)

    raw_lim = CLAMP / SCALE          # |q.k| limit before scaling
    cand_lim = raw_lim * 0.94        # margin for bf16 rounding on device
    hard_lim = raw_lim * 0.97
    for b in range(B):
        qr, kr = qrs[b]
        qn = np.sqrt((qr.astype(np.float64) ** 2).sum(-1))  # [S, H]
        kn = np.sqrt((kr.astype(np.float64) ** 2).sum(-1))
        for h in range(H):
            prod = qn[:, h][:, None] * kn[:, h][None, :]
            if causal:
                # only lower-triangle (k <= q) pairs are live post-mask
                cand = prod >= cand_lim
                ii, jj = np.nonzero(cand)
                keep = jj <= ii
                ii, jj = ii[keep], jj[keep]
            else:
                ii, jj = np.nonzero(prod >= cand_lim)
            if len(ii):
                dots = np.einsum(
                    "nd,nd->n",
                    qr[ii, h].astype(np.float64),
                    kr[jj, h].astype(np.float64),
                )
                if np.abs(dots).max() >= hard_lim:
                    return False
    return True


def kernel(x, mask, Wq, Wk, Wv, Wo):
    global LAST_EXEC_NS
    x = np.asarray(x)
    mask = np.asarray(mask)
    in_dtype = x.dtype

    tril = np.tril(np.ones((S, S), dtype=np.int64))
    m64 = (np.asarray(mask) != 0).astype(np.int64)
    if all((m64[b] == tril).all() for b in range(B)):
        variant = "causal"
    elif (m64 != 0).all():
        variant = "ones"
    else:
        variant = "general"

    # clip-elision guards, proven on the host with margin for bf16 rounding
    no_xclip = bool(np.abs(x).max() < CLAMP * 0.999)
    no_expclip = False
    if variant in ("causal", "ones") and no_xclip:
        no_expclip = _prove_no_expclip(x, Wq, Wk, variant == "causal")

    nc = _get_program(variant, no_xclip, no_expclip)

    cos, sin = _rope_tables()
    cosT = np.ascontiguousarray(cos.T).astype(BF16)   # [DH, S]
    sinMh = np.empty((DH, S), dtype=np.float32)       # sign-folded for rotated q
    sinMh[0:64, :] = -sin.T[0:64, :]                  # row d<64  -> -sin[:, d]
    sinMh[64:128, :] = sin.T[64:128, :]               # row d>=64 -> +sin[:, d]
    sinMh = sinMh.astype(BF16)

    if variant == "causal":
        iu = np.arange(128)[:, None]
        ju = np.arange(896)[None, :]
        bandh = (iu <= ju - 384).astype(BF16)

    in_maps = []
    for c in range(NCORES):
        b, g = divmod(c, 4)
        sl = slice(g * GD, (g + 1) * GD)
        # flat layouts (see _build_program): xTr [128, NSB*NE*SB],
        # w*T [128, NE*GD]
        xr = (
            x[b].T.reshape(NE, 128, NSB, SB).transpose(1, 2, 0, 3).reshape(128, -1)
        )

        def wflat(W):
            wt = np.asarray(W)[sl, :].T  # [D, GD]
            return wt.reshape(NE, 128, GD).transpose(1, 0, 2).reshape(128, -1)

        def wflat_d(W):  # d-major: [p, d*(NE*128) + e*128 + c]
            wt = np.asarray(W)[sl, :].T  # [D, GD]
            return (
                wt.reshape(NE, 128, GH, 128).transpose(1, 2, 0, 3).reshape(128, -1)
            )

        im = {
            "xTr": np.ascontiguousarray(xr).astype(BF16),
            "wqT": np.ascontiguousarray(wflat_d(Wq)).astype(BF16),
            "wkT": np.ascontiguousarray(wflat_d(Wk)).astype(BF16),
            "wvT": np.ascontiguousarray(wflat(Wv)).astype(BF16),
            "woT": np.ascontiguousarray(np.asarray(Wo)[:, sl].T).astype(BF16),
            "cosT": cosT,
            "sinM": sinMh,
        }
        if variant == "causal":
            im["bandT"] = bandh
            selh = np.zeros((128, GD), dtype=np.float32)
            for hh in range(GH):
                selh[32 * hh : 32 * hh + 32, hh * 128 : (hh + 1) * 128] = 1.0 / 32.0
            im["selT"] = selh
        elif variant == "general":
            im["maskT"] = np.ascontiguousarray(m64[b].T).astype(BF16)
        in_maps.append(im)

    kwargs = {}
    if TRACE:
        kwargs["trace"] = True
        if TRACE_DIR:
            kwargs["tmpdir"] = TRACE_DIR
    res = run_bass_kernel_spmd(nc, in_maps, core_ids=list(range(NCORES)), **kwargs)
    LAST_EXEC_NS = res.exec_time_ns
    globals()["LAST_RESULT"] = res

    out = np.zeros((B, S, D), dtype=np.float32)
    for b in range(B):
        acc = np.zeros((S, D), dtype=np.float32)
        for g in range(4):
            acc += res.results[b * 4 + g]["outp"].astype(np.float32)
        out[b] = np.clip(acc, -CLAMP, CLAMP)
    return out.astype(in_dtype, copy=False)


# revision 69
# speedup vs baseline: 1.0125x; 1.0125x over previous
"""Multi-head attention (QKV proj + RoPE + masked softmax + out-proj) on 8 TRN2 cores.

Sharding (tensor-parallel heads x data-parallel batch):
  core c in 0..7  ->  batch b = c // 4, head-group g = c % 4 (heads 4g..4g+3).
Each core computes its 512-wide q/k/v head slice, RoPE, attention for its 4
heads, and a partial output projection  ao_slice @ Wo[:, slice].T  (full [S, D]).
Host sums the 4 partials per batch and applies the final clip.

Device layouts (per core):
  xT    [D, S]  bf16   x[b].T (clipped on device unless host proves |x|<CLAMP)
  wqkvT [D, 3*GD] bf16 concat of Wq/Wk/Wv head-slice transposes; resident in SBUF
  woT   [512, D] bf16  Wo[:, slice].T
  cosT  [128, S] f32; sinM [128, S] f32 (sign/swap-folded rope table)
  q/k kept d-major [128(d), S] per head; v kept s-major [128(s), 512(hd)]
  scores computed transposed [sk, sq] so softmax denom = ones-matmul on PE.

Program order interleaves qkv(j+1) / attn(j) / oproj(j-1) so the Tile
scheduler can fill attention-pipeline stalls with projection/output matmuls.
"""

import os
import sys

if "/opt/trn_rl_repo" not in sys.path:
    sys.path.insert(0, "/opt/trn_rl_repo")
os.environ.setdefault("JAX_PLATFORMS", "")

from contextlib import ExitStack

import ml_dtypes
import numpy as np

import concourse.bass as bass
import concourse.mybir as mybir
import concourse.tile as tile
from concourse import bacc
from concourse.bass_utils import run_bass_kernel_spmd

BF16 = ml_dtypes.bfloat16
B, S, D, H = 2, 2048, 2048, 16
DH = 128
CLAMP = 10.0
SCALE = float(1.0 / np.sqrt(np.float32(DH)))
NCORES = 8
GH = 4            # heads per core
GD = GH * DH      # 512
SB = 512          # s-block width
NSB = S // SB     # 4
NE = D // 128     # 16 contraction chunks
NSK = S // 128    # 16
F32 = mybir.dt.float32
BF = mybir.dt.bfloat16
MIN_ = mybir.AluOpType.min
MAX_ = mybir.AluOpType.max
MULT = mybir.AluOpType.mult
EXP = mybir.ActivationFunctionType.Exp
EXPHI = float(np.exp(np.float32(CLAMP)))
EXPLO = float(np.exp(np.float32(-CLAMP)))

# module-level knobs read by test.py
TRACE = False
TRACE_DIR = None
LAST_EXEC_NS = None
LAST_RESULT = None

_PROGRAMS = {}

# tunables (read at program-build time)
KNOBS = {
    "px": 2,
    "prope": 3,
    "prot": 2,
    "pnrm": 2,
    "po_st": 6,
    "pp": 8,
}


def _build_program(variant, no_xclip=False, no_expclip=False):
    """variant: 'causal' (tril mask), 'ones' (no mask), 'general' (mask tensor).

    no_xclip: host verified max|x| < CLAMP, so clip(x) is an identity.
    no_expclip: host verified (exact candidate-pair check) that all scores in
    the live (unmasked) region stay inside +-CLAMP, so the score clip is an
    identity and exp never saturates."""
    nc = bacc.Bacc(
        "TRN2",
        target_bir_lowering=False,
        debug=False,
        enable_asserts=False,
        num_devices=NCORES,
    )
    # flat layouts so whole tensors load in a few large DMAs:
    #   xTr[p, ((j*NE)+e)*SB + c]        = x.T[e*128+p, j*SB+c]
    #   wq/wkT[p, d*(NE*128) + e*128 + c] = W.T[e*128+p, d*128+c]  (d-major: one
    #     512KB slice per q/k chain, so chain d can start before slice d+1 lands)
    #   wvT[p, e*GD + c]                 = Wv.T[e*128+p, c]
    xTr = nc.dram_tensor("xTr", [128, NSB * NE * SB], BF, kind="ExternalInput")
    wqT = nc.dram_tensor("wqT", [128, NE * GD], BF, kind="ExternalInput")
    wkT = nc.dram_tensor("wkT", [128, NE * GD], BF, kind="ExternalInput")
    wvT = nc.dram_tensor("wvT", [128, NE * GD], BF, kind="ExternalInput")
    woT = nc.dram_tensor("woT", [GD, D], BF, kind="ExternalInput")
    cosT = nc.dram_tensor("cosT", [DH, S], BF, kind="ExternalInput")
    sinM = nc.dram_tensor("sinM", [DH, S], BF, kind="ExternalInput")
    bandT = maskT = selT = None
    if variant == "causal":
        bandT = nc.dram_tensor("bandT", [128, 896], BF, kind="ExternalInput")
        # selT[p, h*128+o] = 1/32 if 32h <= p < 32h+32 else 0 — broadcast matrix
        # that expands the col-tiled denominators to all 128 partitions
        selT = nc.dram_tensor("selT", [128, GD], F32, kind="ExternalInput")
    elif variant == "general":
        maskT = nc.dram_tensor("maskT", [S, S], BF, kind="ExternalInput")
    outp = nc.dram_tensor("outp", [S, D], BF, kind="ExternalOutput")

    with ExitStack() as ctx:
        tc = ctx.enter_context(tile.TileContext(nc))
        p_x = ctx.enter_context(tc.tile_pool(name="px", bufs=KNOBS["px"]))
        p_w = ctx.enter_context(tc.tile_pool(name="pw", bufs=1))
        p_qk = ctx.enter_context(tc.tile_pool(name="pqk", bufs=2 * GH))
        p_v = ctx.enter_context(tc.tile_pool(name="pv", bufs=NSK))
        p_rope = ctx.enter_context(tc.tile_pool(name="prope", bufs=KNOBS["prope"]))
        p_rot = ctx.enter_context(tc.tile_pool(name="prot", bufs=KNOBS["prot"]))
        p_tab = ctx.enter_context(tc.tile_pool(name="ptab", bufs=1))
        p_p = ctx.enter_context(tc.tile_pool(name="pp", bufs=KNOBS["pp"]))
        p_ao = ctx.enter_context(tc.tile_pool(name="pao", bufs=GH))
        p_nrm = ctx.enter_context(tc.tile_pool(name="pnrm", bufs=KNOBS["pnrm"]))
        p_wo = ctx.enter_context(tc.tile_pool(name="pwo", bufs=GH))
        p_o = ctx.enter_context(tc.tile_pool(name="po_st", bufs=KNOBS["po_st"]))
        # PSUM: 8 banks split per phase/tag so cross-phase interleave never
        # serializes on a shared slot ring. bufs is per-tag: 2+2+1+2+1 = 8.
        p_ps_q = ctx.enter_context(tc.tile_pool(name="ppsq", bufs=2, space="PSUM"))
        p_ps_po = ctx.enter_context(tc.tile_pool(name="ppspo", bufs=2, space="PSUM"))
        p_ps_pd = ctx.enter_context(tc.tile_pool(name="ppspd", bufs=1, space="PSUM"))
        p_ps_sc = ctx.enter_context(tc.tile_pool(name="ppssc", bufs=2, space="PSUM"))
        p_ps_o = ctx.enter_context(tc.tile_pool(name="ppso", bufs=1, space="PSUM"))
        if variant == "general":
            p_m = ctx.enter_context(tc.tile_pool(name="pm", bufs=NSK + 4))

        dmaq = [nc.sync, nc.scalar, nc.gpsimd]

        # persistent SBUF tensors (flat: slice [:, e*GD+...] per e-chunk)
        wqr = p_w.tile([128, NE * GD], BF, tag="wq", name="wqr")
        wkr = p_w.tile([128, NE * GD], BF, tag="wk", name="wkr")
        wvr = p_w.tile([128, NE * GD], BF, tag="wv", name="wvr")
        qbf = [p_qk.tile([128, S], BF, tag="qk", name=f"qbf{i}") for i in range(GH)]
        kbf = [p_qk.tile([128, S], BF, tag="qk", name=f"kbf{i}") for i in range(GH)]
        aobf = [p_ao.tile([128, S], BF, tag="ao", name=f"aobf{i}") for i in range(GH)]
        vbf = [None] * NSK
        xts = [None] * NSB
        wot = []

        def x_load(j, q, widths=None):
            t = p_x.tile([128, NE * SB], BF, tag="x", name=f"xt{j}")
            widths = widths or [NE * SB]
            off = 0
            for w in widths:
                q.dma_start(
                    out=t[:, off : off + w],
                    in_=xTr[:, j * NE * SB + off : j * NE * SB + off + w],
                )
                off += w
            assert off == NE * SB
            if not no_xclip:
                nc.gpsimd.tensor_scalar(t, t, CLAMP, -CLAMP, MIN_, MAX_)
            xts[j] = t

        # ---------------- QKV projections + RoPE ----------------
        # rings: list of (pool, tag) cycled for the chain PSUM tiles. Blocks
        # that don't overlap attention borrow its idle banks for deeper
        # drain pipelining.
        def qkv_block(j, rings):
            cs = slice(j * SB, (j + 1) * SB)
            xt = xts[j]
            ci = 0
            # q, k: d-major [128(d=head), s 512]; one PSUM chain per head
            for wbuf, dstbuf in ((wqr, qbf), (wkr, kbf)):
                for d_ in range(GH):
                    pool, ptag = rings[ci % len(rings)]
                    ci += 1
                    ps = pool.tile([128, SB], F32, tag=ptag, name="psqk")
                    d0 = d_ * NE * 128
                    for e in range(NE):
                        nc.tensor.matmul(
                            ps,
                            lhsT=wbuf[:, d0 + e * 128 : d0 + (e + 1) * 128],
                            rhs=xt[:, e * SB : (e + 1) * SB],
                            start=(e == 0),
                            stop=(e == NE - 1),
                        )
                    # drain+clip on DVE (frees the PSUM bank fast); rope math
                    # in bf16 for 2x DVE rate
                    qc = p_rope.tile([128, SB], BF, tag="r1", name="qc")
                    nc.vector.tensor_scalar(qc, ps, CLAMP, -CLAMP, MIN_, MAX_)
                    # rotate_half via DMA (cross-partition moves are free on DMA);
                    # keep these off the scalar queue so ACT stays free for exp
                    qcr = p_rot.tile([128, SB], BF, tag="rot", name="qcr")
                    rq = nc.sync if d_ % 2 else nc.gpsimd
                    rq.dma_start(out=qcr[0:64, :], in_=qc[64:128, :])
                    rq.dma_start(out=qcr[64:128, :], in_=qc[0:64, :])
                    t2 = p_rope.tile([128, SB], BF, tag="r2", name="t2")
                    nc.vector.tensor_tensor(t2, qcr, sinm[:, cs], MULT)
                    qp = p_rope.tile([128, SB], BF, tag="r3", name="qp")
                    nc.vector.tensor_tensor(qp, qc, cosc[:, cs], MULT)
                    nc.vector.tensor_add(qp, qp, t2)
                    nc.gpsimd.tensor_scalar(
                        dstbuf[d_][:, cs], qp, CLAMP, -CLAMP, MIN_, MAX_
                    )

            # v: s-major [s_tile 128, hd 512]
            for st in range(4):
                pool, ptag = rings[ci % len(rings)]
                ci += 1
                vps = pool.tile([128, GD], F32, tag=ptag, name="vps")
                for e in range(NE):
                    nc.tensor.matmul(
                        vps,
                        lhsT=xt[:, e * SB + st * 128 : e * SB + (st + 1) * 128],
                        rhs=wvr[:, e * GD : (e + 1) * GD],
                        start=(e == 0),
                        stop=(e == NE - 1),
                    )
                vt = p_v.tile([128, GD], BF, tag="v", name=f"vt{j}_{st}")
                nc.vector.tensor_scalar(vt, vps, CLAMP, -CLAMP, MIN_, MAX_)
                vbf[j * 4 + st] = vt

        # ---------------- attention ----------------
        def attn_block(j):
            cs = slice(j * SB, (j + 1) * SB)
            nsk = 4 * j + 4 if variant == "causal" else NSK
            mts = None
            if variant == "general":
                mts = []
                for sk in range(NSK):
                    mt = p_m.tile([128, SB], BF, tag="m", name=f"mt{sk}")
                    nc.sync.dma_start(
                        out=mt, in_=maskT[sk * 128 : (sk + 1) * 128, cs]
                    )
                    mts.append(mt)
            for h in range(GH):
                po = p_ps_po.tile([128, SB], F32, tag="po", name="po")
                pd = p_ps_pd.tile([128, SB], F32, tag="pd", name="pd")
                for sk in range(nsk):
                    # causal diag tiles: columns < 128*r are fully masked; skip them
                    c0 = 0
                    if variant == "causal" and sk >= 4 * j:
                        c0 = 128 * (sk - 4 * j)
                    w_ = SB - c0
                    psc = p_ps_sc.tile([128, SB], F32, tag="psc", name="psc")
                    nc.tensor.matmul(
                        psc[:, c0:SB],
                        lhsT=kbf[h][:, sk * 128 : (sk + 1) * 128],
                        rhs=qbf[h][:, j * SB + c0 : (j + 1) * SB],
                        start=True,
                        stop=True,
                    )
                    pt = p_p.tile([128, SB], BF, tag="p", name="pt")
                    nc.scalar.activation(pt[:, c0:SB], psc[:, c0:SB], EXP, scale=SCALE)
                    # post-exp clip == exp of pre-clipped score (exp is monotone;
                    # ACT exp saturates to inf/0 which min/max maps to exp(+-10)).
                    if variant == "general":
                        nc.vector.tensor_scalar(
                            pt[:, c0:SB], pt[:, c0:SB], EXPHI, EXPLO, MIN_, MAX_
                        )
                        nc.vector.tensor_tensor(pt, pt, mts[sk], MULT)
                    elif variant == "causal" and sk >= 4 * j:
                        if no_expclip:
                            nc.vector.tensor_tensor(
                                pt[:, c0:SB], pt[:, c0:SB],
                                band[:, 384 : 384 + w_], MULT,
                            )
                        else:
                            nc.vector.scalar_tensor_tensor(
                                pt[:, c0:SB], pt[:, c0:SB], EXPHI,
                                band[:, 384 : 384 + w_], MIN_, MULT,
                            )
                    elif not no_expclip:
                        nc.vector.tensor_scalar(
                            pt[:, c0:SB], pt[:, c0:SB], EXPHI, EXPLO, MIN_, MAX_
                        )
                    nc.tensor.matmul(
                        po[:, c0:SB],
                        lhsT=vbf[sk][:, h * 128 : (h + 1) * 128],
                        rhs=pt[:, c0:SB],
                        start=(sk == 0),
                        stop=(sk == nsk - 1),
                    )
                    nc.tensor.matmul(
                        pd[:, c0:SB],
                        lhsT=ones,
                        rhs=pt[:, c0:SB],
                        start=(sk == 0),
                        stop=(sk == nsk - 1),
                    )
                # pd rows are all identical (= softmax denominator broadcast)
                rcb = p_nrm.tile([128, SB], F32, tag="rcb", name="rcb")
                nc.vector.reciprocal_approx_fast(rcb, pd)
                a32 = p_nrm.tile([128, SB], F32, tag="a32", name="a32")
                nc.vector.tensor_tensor(a32, po, rcb, MULT)
                nc.gpsimd.tensor_scalar(
                    aobf[h][:, cs], a32, CLAMP, -CLAMP, MIN_, MAX_
                )

        def attn_block_ct(j):
            # causal-only; runs when no qkv block overlaps (PSUM free): sk-outer,
            # heads inner; the 4 heads' softmax denominators ride one PSUM bank
            # via col-tiled ones-matmuls (concurrent 32-col groups), then a
            # broadcast matmul expands 1/d per head to 128 partitions.
            cs = slice(j * SB, (j + 1) * SB)
            nsk = 4 * j + 4
            po_pairs = [
                (p_ps_po, "po"), (p_ps_po, "po"), (p_ps_q, "ps"), (p_ps_q, "ps"),
            ]
            pos = [
                pool.tile([128, SB], F32, tag=ptag, name=f"poct{h}")
                for h, (pool, ptag) in enumerate(po_pairs)
            ]
            pd4 = p_ps_pd.tile([128, SB], F32, tag="pd", name="pd4")
            for sk in range(nsk):
                c0 = 128 * (sk - 4 * j) if sk >= 4 * j else 0
                w_ = SB - c0
                pts = []
                for h in range(GH):
                    psc = p_ps_sc.tile([128, SB], F32, tag="psc", name="psc")
                    nc.tensor.matmul(
                        psc[:, c0:SB],
                        lhsT=kbf[h][:, sk * 128 : (sk + 1) * 128],
                        rhs=qbf[h][:, j * SB + c0 : (j + 1) * SB],
                        start=True,
                        stop=True,
                    )
                    pt = p_p.tile([128, SB], BF, tag="p", name="pt")
                    nc.scalar.activation(pt[:, c0:SB], psc[:, c0:SB], EXP, scale=SCALE)
                    if sk >= 4 * j:
                        if no_expclip:
                            nc.vector.tensor_tensor(
                                pt[:, c0:SB], pt[:, c0:SB],
                                band[:, 384 : 384 + w_], MULT,
                            )
                        else:
                            nc.vector.scalar_tensor_tensor(
                                pt[:, c0:SB], pt[:, c0:SB], EXPHI,
                                band[:, 384 : 384 + w_], MIN_, MULT,
                            )
                    elif not no_expclip:
                        nc.vector.tensor_scalar(
                            pt[:, c0:SB], pt[:, c0:SB], EXPHI, EXPLO, MIN_, MAX_
                        )
                    nc.tensor.matmul(
                        pos[h][:, c0:SB],
                        lhsT=vbf[sk][:, h * 128 : (h + 1) * 128],
                        rhs=pt[:, c0:SB],
                        start=(sk == 0),
                        stop=(sk == nsk - 1),
                    )
                    pts.append(pt)
                # 4 col-tiled denominator matmuls back-to-back -> concurrent
                # 32-col groups. Only the very first clears the bank; later
                # groups' first writes land on cleared has_written bits and
                # overwrite, so every group accumulates correctly.
                for h in range(GH):
                    nc.tensor.matmul(
                        pd4[32 * h : 32 * h + 32, c0:SB],
                        lhsT=ones[:, 0:32],
                        rhs=pts[h][:, c0:SB],
                        start=(sk == 0 and h == 0),
                        stop=(sk == nsk - 1 and h == GH - 1),
                        tile_position=(0, 32 * h),
                        skip_group_check=True,
                    )
            rcp4 = p_nrm.tile([128, SB], F32, tag="rcb", name="rcp4")
            nc.vector.reciprocal_approx_fast(rcp4, pd4)
            for h in range(GH):
                rcbps = p_ps_sc.tile([128, SB], F32, tag="psc", name="rcbps")
                nc.tensor.matmul(
                    rcbps,
                    lhsT=selc[:, h * 128 : (h + 1) * 128],
                    rhs=rcp4,
                    start=True,
                    stop=True,
                )
                rcb = p_nrm.tile([128, SB], F32, tag="a32", name="rcbsb")
                nc.scalar.copy(rcb, rcbps)
                a32 = p_nrm.tile([128, SB], F32, tag="a32", name="a32")
                nc.vector.tensor_tensor(a32, pos[h], rcb, MULT)
                nc.gpsimd.tensor_scalar(
                    aobf[h][:, cs], a32, CLAMP, -CLAMP, MIN_, MAX_
                )

        # -------- output projection (partial over this head slice) --------
        def oproj_block(j, rings, use_act=True):
            # use_act=False while attention runs: ACT must stay free for exp
            ci = 0
            for sq in range(4 * j, 4 * j + 4):
                for eb in range(NSB):
                    pfpool, pftag = rings[ci % len(rings)]
                    ci += 1
                    pf = pfpool.tile([128, SB], F32, tag=pftag, name="pf")
                    for h in range(GH):
                        nc.tensor.matmul(
                            pf,
                            lhsT=aobf[h][:, sq * 128 : (sq + 1) * 128],
                            rhs=wot[h][:, eb * SB : (eb + 1) * SB],
                            start=(h == 0),
                            stop=(h == GH - 1),
                        )
                    ot = p_o.tile([128, SB], BF, tag="ot", name="ot")
                    if use_act and eb % 2 == 0:
                        nc.scalar.copy(ot, pf)
                    else:
                        nc.vector.tensor_copy(ot, pf)
                    nc.sync.dma_start(
                        out=outp[sq * 128 : (sq + 1) * 128, eb * SB : (eb + 1) * SB],
                        in_=ot,
                    )

        # ---------------- program order ----------------
        # HBM transfers share bandwidth round-robin once issued, so strict
        # per-queue FIFO sequencing IS the priority mechanism: critical-path
        # loads (wq, x0) head their queues; deferrables queue strictly behind.
        #   sync:   wq d0..d3 | wk d0..d3 | x2 | outputs
        #   gpsimd: x0 c0..c3 | wv c0..c3 | x1 | wo | x3
        #   scalar: cos sin band (small, needed ~15us in)
        NC4 = 4
        cw = NE * GD // NC4
        # progressive chunks: first pieces small so the first chain's e=0
        # matmul fires as early as possible
        for c0, c1 in ((0, cw // 4), (cw // 4, cw // 2), (cw // 2, cw)):
            nc.sync.dma_start(out=wqr[:, c0:c1], in_=wqT[:, c0:c1])
        for c in range(1, NC4):
            nc.sync.dma_start(
                out=wqr[:, c * cw : (c + 1) * cw], in_=wqT[:, c * cw : (c + 1) * cw]
            )
        x_load(0, nc.gpsimd, widths=[512, 512, 1024, 2048, 4096])

        # wk_d0 right behind wq on sync (k-chains need it ~24us in); rope
        # tables woven after it (consumed a few us later); then the rest of
        # wk. scalar stays clear so only two streams share HBM early.
        nc.sync.dma_start(out=wkr[:, 0:cw], in_=wkT[:, 0:cw])
        cosc = p_tab.tile([DH, S], BF, tag="cos")
        nc.sync.dma_start(out=cosc, in_=cosT[:, :])
        sinm = p_tab.tile([DH, S], BF, tag="sin")
        nc.sync.dma_start(out=sinm, in_=sinM[:, :])
        ones = p_tab.tile([128, 128], BF, tag="ones")
        nc.vector.memset(ones, 1.0)
        for c in range(1, NC4):
            nc.sync.dma_start(
                out=wkr[:, c * cw : (c + 1) * cw], in_=wkT[:, c * cw : (c + 1) * cw]
            )
        band = selc = None
        if variant == "causal":
            band = p_tab.tile([128, 896], BF, tag="band")
            nc.sync.dma_start(out=band, in_=bandT[:, :])
            selc = p_tab.tile([128, GD], F32, tag="sel")
            nc.sync.dma_start(out=selc, in_=selT[:, :])
        for c in range(NC4):
            nc.gpsimd.dma_start(
                out=wvr[:, c * cw : (c + 1) * cw], in_=wvT[:, c * cw : (c + 1) * cw]
            )
        x_load(1, nc.gpsimd, widths=[4096, 4096])

        wide = [(p_ps_q, "ps"), (p_ps_sc, "psc"), (p_ps_po, "po")]
        narrow = [(p_ps_q, "ps")]
        qkv_block(0, wide)
        qkv_block(1, wide)
        attn_block(1)
        for hh in range(GH):
            t = p_wo.tile([128, D], BF, tag="wo", name=f"wot{hh}")
            nc.gpsimd.dma_start(out=t, in_=woT[hh * 128 : (hh + 1) * 128, :])
            wot.append(t)
        x_load(2, nc.sync)
        qkv_block(2, narrow)
        attn_block(2)
        oproj_block(1, [(p_ps_o, "pf")])
        x_load(3, nc.gpsimd)
        qkv_block(3, narrow)
        attn_block(3)
        oproj_block(2, [(p_ps_o, "pf")])
        attn_block(0)
        # qkv ring is idle post-qkv: 2-bank filler for attn(0)'s exp stalls
        oproj_block(3, [(p_ps_q, "ps"), (p_ps_q, "ps")])
        # everything else is drained by now: dense multi-bank tail
        oproj_block(0, [(p_ps_q, "ps"), (p_ps_o, "pf"), (p_ps_q, "ps")])

    nc.compile()
    return nc


def _get_program(variant, no_xclip=False, no_expclip=False):
    key = (variant, no_xclip, no_expclip, tuple(sorted(KNOBS.items())))
    if key not in _PROGRAMS:
        _PROGRAMS[key] = _build_program(variant, no_xclip, no_expclip)
    return _PROGRAMS[key]


def _rope_tables():
    inv_freq = 1.0 / (10000.0 ** (np.arange(0, DH, 2, dtype=np.float32) / np.float32(DH)))
    pos = np.arange(S, dtype=np.float32)
    freqs = pos[:, None] * inv_freq[None, :]          # [S, DH/2]
    emb = np.concatenate([freqs, freqs], axis=-1)     # [S, DH]
    return np.cos(emb).astype(np.float32), np.sin(emb).astype(np.float32)


def _rot(t):
    return np.concatenate([-t[..., 64:], t[..., :64]], axis=-1)


def _prove_no_expclip(x, Wq, Wk, causal):
    """Exact host-side proof that no live score reaches +-CLAMP.

    Returns True only if, for every unmasked (q,k) pair, |q.k|*SCALE stays
    below CLAMP with margin for device-side bf16 rounding of x/W/q/k.
    Candidate pairs are prefiltered by the Cauchy-Schwarz norm product and
    then checked with exact dot products."""
    cos_h, sin_h = _rope_tables()
    lim = CLAMP * 0.999
    qrs = []
    for b in range(B):
        xb = x[b].astype(np.float32)
        pair = []
        for W in (Wq, Wk):
            qh = xb @ np.asarray(W, dtype=np.float32).T
            if np.abs(qh).max() >= lim:
                return False  # pre-rope clip binds; bail conservatively
            qh = qh.reshape(S, H, DH)
            qr = qh * cos_h[:, None, :] + _rot(qh) * sin_h[:, None, :]
            if np.abs(qr).max() >= lim:
                return False  # post-rope clip binds; bail conservatively
            pair.append(qr)
        qrs.append(pair)

    raw_lim = CLAMP / SCALE          # |q.k| limit before scaling
    cand_lim = raw_lim * 0.94        # margin for bf16 rounding on device
    hard_lim = raw_lim * 0.97
    for b in range(B):
        qr, kr = qrs[b]
        qn = np.sqrt((qr.astype(np.float64) ** 2).sum(-1))  # [S, H]
        kn = np.sqrt((kr.astype(np.float64) ** 2).sum(-1))
        for h in range(H):
            prod = qn[:, h][:, None] * kn[:, h][None, :]
            if causal:
                # only lower-triangle (k <= q) pairs are live post-mask
                cand = prod >= cand_lim
                ii, jj = np.nonzero(cand)
                keep = jj <= ii
                ii, jj = ii[keep], jj[keep]
            else:
                ii, jj = np.nonzero(prod >= cand_lim)
            if len(ii):
                dots = np.einsum(
                    "nd,nd->n",
                    qr[ii, h].astype(np.float64),
                    kr[jj, h].astype(np.float64),
                )
                if np.abs(dots).max() >= hard_lim:
                    return False
    return True


def kernel(x, mask, Wq, Wk, Wv, Wo):
    global LAST_EXEC_NS
    x = np.asarray(x)
    mask = np.asarray(mask)
    in_dtype = x.dtype

    tril = np.tril(np.ones((S, S), dtype=np.int64))
    m64 = (np.asarray(mask) != 0).astype(np.int64)
    if all((m64[b] == tril).all() for b in range(B)):
        variant = "causal"
    elif (m64 != 0).all():
        variant = "ones"
    else:
        variant = "general"

    # clip-elision guards, proven on the host with margin for bf16 rounding
    no_xclip = bool(np.abs(x).max() < CLAMP * 0.999)
    no_expclip = False
    if variant in ("causal", "ones") and no_xclip:
        no_expclip = _prove_no_expclip(x, Wq, Wk, variant == "causal")

    nc = _get_program(variant, no_xclip, no_expclip)

    cos, sin = _rope_tables()
    cosT = np.ascontiguousarray(cos.T).astype(BF16)   # [DH, S]
    sinMh = np.empty((DH, S), dtype=np.float32)       # sign-folded for rotated q
    sinMh[0:64, :] = -sin.T[0:64, :]                  # row d<64  -> -sin[:, d]
    sinMh[64:128, :] = sin.T[64:128, :]               # row d>=64 -> +sin[:, d]
    sinMh = sinMh.astype(BF16)

    if variant == "causal":
        iu = np.arange(128)[:, None]
        ju = np.arange(896)[None, :]
        bandh = (iu <= ju - 384).astype(BF16)

    in_maps = []
    for c in range(NCORES):
        b, g = divmod(c, 4)
        sl = slice(g * GD, (g + 1) * GD)
        # flat layouts (see _build_program): xTr [128, NSB*NE*SB],
        # w*T [128, NE*GD]
        xr = (
            x[b].T.reshape(NE, 128, NSB, SB).transpose(1, 2, 0, 3).reshape(128, -1)
        )

        def wflat(W):
            wt = np.asarray(W)[sl, :].T  # [D, GD]
            return wt.reshape(NE, 128, GD).transpose(1, 0, 2).reshape(128, -1)

        def wflat_d(W):  # d-major: [p, d*(NE*128) + e*128 + c]
            wt = np.asarray(W)[sl, :].T  # [D, GD]
            return (
                wt.reshape(NE, 128, GH, 128).transpose(1, 2, 0, 3).reshape(128, -1)
            )

        im = {
            "xTr": np.ascontiguousarray(xr).astype(BF16),
            "wqT": np.ascontiguousarray(wflat_d(Wq)).astype(BF16),
            "wkT": np.ascontiguousarray(wflat_d(Wk)).astype(BF16),
            "wvT": np.ascontiguousarray(wflat(Wv)).astype(BF16),
            "woT": np.ascontiguousarray(np.asarray(Wo)[:, sl].T).astype(BF16),
            "cosT": cosT,
            "sinM": sinMh,
        }
        if variant == "causal":
            im["bandT"] = bandh
            selh = np.zeros((128, GD), dtype=np.float32)
            for hh in range(GH):
                selh[32 * hh : 32 * hh + 32, hh * 128 : (hh + 1) * 128] = 1.0 / 32.0
            im["selT"] = selh
        elif variant == "general":
            im["maskT"] = np.ascontiguousarray(m64[b].T).astype(BF16)
        in_maps.append(im)

    kwargs = {}
    if TRACE:
        kwargs["trace"] = True
        if TRACE_DIR:
            kwargs["tmpdir"] = TRACE_DIR
    res = run_bass_kernel_spmd(nc, in_maps, core_ids=list(range(NCORES)), **kwargs)
    LAST_EXEC_NS = res.exec_time_ns
    globals()["LAST_RESULT"] = res

    out = np.zeros((B, S, D), dtype=np.float32)
    for b in range(B):
        acc = np.zeros((S, D), dtype=np.float32)
        for g in range(4):
            acc += res.results[b * 4 + g]["outp"].astype(np.float32)
        out[b] = np.clip(acc, -CLAMP, CLAMP)
    return out.astype(in_dtype, copy=False)
